# revision 8
# baseline (speedup 1.0000x reference)
"""Trainium2 Bass kernel for ClipPairWiseLossAll.

loss = sum_{i<j} || relu(r_i - r_j) ||_2   with r = repr[GT], M=512, N=768.

Strategy (8 NeuronCores, SPMD, single NEFF):
  * Host: gather r = repr[GT], transpose -> rT [N=768, M=512], cast bf16.
  * Pair space (t, s), t < s; the s axis is split across cores with a folded
    assignment (core c gets s in {16k+c, 16k+15-c}) -> 16352 pairs/core.
  * rT lives in SBUF as 6 chunks of [128 n-partitions, 512 t-free]. Per s:
    E[n, t] = relu(r[t,n] - r[s,n]) via tensor_scalar (per-partition scalar
    = column rT[:, s]; DVE 4x bf16 mode), some k's produced on GPSIMD.
    E2 = E*E in fp8 (ACT Square, two s per instruction).
    fp8 DoubleRow one-hot matmuls contract E2 over n into PSUM row m_s
    (3 matmuls per s, 2 chunks each).
  * t ranges rounded up to L_k = 16(k+1) (identical shapes on all cores ->
    one NEFF); a per-core mask kills invalid columns, ACT computes sqrt with
    fused row-sum, host adds the 8x64 partials.
"""

import numpy as np

M = 512
N = 768
P = 128
NCH = N // P  # 6
NCORES = 8
NS = 64  # s values per core


def _s_list(c):
    out = []
    for k in range(32):
        out.append(16 * k + c)
        out.append(16 * k + 15 - c)
    return out


_PROG = {}


def PROD_ENGINE(k, slot):
    # producer split: GPSIMD takes some mid/large k's, DVE the rest
    return "gps" if k % 5 == 2 else "dve"


def _build_program():
    if "nc" in _PROG:
        return _PROG["nc"]

    from contextlib import ExitStack

    import concourse.bacc as bacc
    import concourse.tile as tile
    from concourse import mybir

    AOT = mybir.AluOpType
    AFT = mybir.ActivationFunctionType
    bf16 = mybir.dt.bfloat16
    fp8 = mybir.dt.float8e4
    f32 = mybir.dt.float32

    nc = bacc.Bacc(
        "TRN2",
        target_bir_lowering=False,
        debug=False,
        enable_asserts=False,
        num_devices=NCORES,
    )

    rt_d = nc.dram_tensor("rt", [N, M], bf16, kind="ExternalInput")
    sc_d = nc.dram_tensor("sc", [N, NS], f32, kind="ExternalInput")
    mk_d = nc.dram_tensor("mk", [NS, M], f32, kind="ExternalInput")
    oh_d = nc.dram_tensor("oh", [P, NS * 2 * NS], fp8, kind="ExternalInput")
    out_d = nc.dram_tensor("out", [NS, 1], f32, kind="ExternalOutput")

    with ExitStack() as ctx:
        tc = ctx.enter_context(tile.TileContext(nc))
        singles = ctx.enter_context(tc.tile_pool(name="singles", bufs=1))
        epool = ctx.enter_context(tc.tile_pool(name="e", bufs=3))
        e2pool = ctx.enter_context(tc.tile_pool(name="e2", bufs=3))
        pspool = ctx.enter_context(tc.tile_pool(name="ps", bufs=1, space="PSUM"))

        rt_sb = singles.tile([P, NCH, M], bf16)
        nc.sync.dma_start(out=rt_sb, in_=rt_d.ap().rearrange("(c p) i -> p c i", p=P))
        sc_sb = singles.tile([P, NCH, NS], f32)
        nc.sync.dma_start(out=sc_sb, in_=sc_d.ap().rearrange("(c p) s -> p c s", p=P))
        mk_sb = singles.tile([NS, M], f32)
        nc.sync.dma_start(out=mk_sb, in_=mk_d.ap())

        # fp8 one-hot lhsT stack (host-built, aligned): oh[:, m, :, :] is a
        # [128, 2, 64] dual-row lhsT whose ones-column lands psum row m.
        oh = singles.tile([P, NS, 2, NS], fp8)
        nc.sync.dma_start(out=oh, in_=oh_d.ap())

        ps = pspool.tile([NS, M], f32)
        nc.vector.memset(ps, 0.0)

        for k in range(32):
            L = 16 * (k + 1)
            e_t = epool.tile([P, 2, NCH, M], bf16, tag="e")
            for slot in range(2):
                m = 2 * k + slot
                peng = nc.gpsimd if PROD_ENGINE(k, slot) == "gps" else nc.vector
                for c in range(NCH):
                    peng.tensor_scalar(
                        out=e_t[:, slot, c, 0:L],
                        in0=rt_sb[:, c, 0:L],
                        scalar1=sc_sb[:, c, m : m + 1],
                        scalar2=0.0,
                        op0=AOT.subtract,
                        op1=AOT.max,
                    )
            e2_t = e2pool.tile([P, 2, NCH, M], fp8, tag="e2")
            nc.scalar.activation(
                out=e2_t[:, :, :, 0:L],
                in_=e_t[:, :, :, 0:L],
                func=AFT.Square,
            )
            for slot in range(2):
                m = 2 * k + slot
                for c2 in range(NCH // 2):
                    nc.tensor.matmul(
                        ps[:, 0:L],
                        oh[:, m, :, :],
                        e2_t[:, slot, 2 * c2 : 2 * c2 + 2, 0:L],
                        start=False,
                        stop=False,
                        skip_group_check=True,
                        perf_mode=mybir.MatmulPerfMode.DoubleRow,
                    )

        masked = singles.tile([NS, M], f32)
        nc.vector.tensor_mul(masked, ps[:, :], mk_sb)
        sqrt_t = singles.tile([NS, M], bf16)
        res = singles.tile([NS, 1], f32)
        nc.scalar.activation(out=sqrt_t, in_=masked, func=AFT.Sqrt, accum_out=res)
        nc.sync.dma_start(out=out_d.ap(), in_=res)

    nc.compile()
    _PROG["nc"] = nc
    return nc


def _in_maps(repr_np, GT_np):
    import ml_dtypes

    r = np.asarray(repr_np, dtype=np.float32)[np.asarray(GT_np).astype(np.int64)]
    rT = np.ascontiguousarray(r.T)  # [N, M] f32
    rT_bf = rT.astype(ml_dtypes.bfloat16)

    ohs = np.zeros((P, NS, 2, NS), dtype=ml_dtypes.float8_e4m3)
    for m in range(NS):
        ohs[:, m, :, m] = 1.0
    ohs = ohs.reshape(P, NS * 2 * NS)

    maps = []
    t_idx = np.arange(M)[None, :]
    for c in range(NCORES):
        s = np.array(_s_list(c))
        sc = np.ascontiguousarray(rT_bf[:, s].astype(np.float32))  # [N, 64]
        mk = (t_idx < s[:, None]).astype(np.float32)  # [64, M]
        maps.append({"rt": rT_bf, "sc": sc, "mk": mk, "oh": ohs})
    return maps


def run_device(repr_np, GT_np, trace=False, trace_cores=None):
    """Run the bass kernel on 8 cores; returns (total, BassKernelResults)."""
    from concourse.bass_utils import run_bass_kernel_spmd

    nc = _build_program()
    maps = _in_maps(repr_np, GT_np)
    res = run_bass_kernel_spmd(
        nc,
        maps,
        core_ids=list(range(NCORES)),
        trace=trace,
        trace_cores=trace_cores,
    )
    total = 0.0
    for core_out in res.results:
        total += float(core_out["out"].astype(np.float64).sum())
    return np.float32(total), res


def kernel(repr, GT):
    total, _ = run_device(repr, GT, trace=False)
    return total


# revision 9
# speedup vs baseline: 3.3403x; 3.3403x over previous
"""Trainium2 Bass kernel for ClipPairWiseLossAll.

loss = sum_{i<j} || relu(r_i - r_j) ||_2   with r = repr[GT], M=512, N=768.

Strategy (8 NeuronCores, SPMD, single NEFF):
  * Host: gather r = repr[GT], transpose -> rT [N=768, M=512], cast bf16.
  * Pair space (t, s), t < s; the s axis is split across cores with a folded
    assignment (core c gets s in {16k+c, 16k+15-c}) -> 16352 pairs/core.
  * rT lives in SBUF as 6 chunks of [128 n-partitions, 512 t-free]. Per s:
    E[n, t] = relu(r[t,n] - r[s,n]) via tensor_scalar (per-partition scalar
    = column rT[:, s]; DVE 4x bf16 mode), some k's produced on GPSIMD.
    E2 = E*E in fp8 (ACT Square, two s per instruction).
    fp8 DoubleRow one-hot matmuls contract E2 over n into PSUM row m_s
    (3 matmuls per s, 2 chunks each).
  * t ranges rounded up to L_k = 16(k+1) (identical shapes on all cores ->
    one NEFF); a per-core mask kills invalid columns, ACT computes sqrt with
    fused row-sum, host adds the 8x64 partials.
"""

import numpy as np

M = 512
N = 768
P = 128
NCH = N // P  # 6
NCORES = 8
NS = 64  # s values per core


def _s_list(c):
    out = []
    for k in range(32):
        out.append(16 * k + c)
        out.append(16 * k + 15 - c)
    return out


_PROG = {}


def PROD_ENGINE(k, slot):
    # GPSIMD is unusable here: it shares SBUF ports with DVE and throttles it
    return "dve"


def _build_program():
    if "nc" in _PROG:
        return _PROG["nc"]

    from contextlib import ExitStack

    import concourse.bacc as bacc
    import concourse.tile as tile
    from concourse import mybir

    AOT = mybir.AluOpType
    AFT = mybir.ActivationFunctionType
    bf16 = mybir.dt.bfloat16
    fp8 = mybir.dt.float8e4
    f32 = mybir.dt.float32

    nc = bacc.Bacc(
        "TRN2",
        target_bir_lowering=False,
        debug=False,
        enable_asserts=False,
        num_devices=NCORES,
    )

    rt_d = nc.dram_tensor("rt", [N, M], bf16, kind="ExternalInput")
    sc_d = nc.dram_tensor("sc", [N, NS], f32, kind="ExternalInput")
    mk_d = nc.dram_tensor("mk", [NS, M], f32, kind="ExternalInput")
    oh_d = nc.dram_tensor("oh", [P, NS * 2 * NS], fp8, kind="ExternalInput")
    out_d = nc.dram_tensor("out", [NS, 1], f32, kind="ExternalOutput")

    with ExitStack() as ctx:
        tc = ctx.enter_context(tile.TileContext(nc))
        singles = ctx.enter_context(tc.tile_pool(name="singles", bufs=1))
        epool = ctx.enter_context(tc.tile_pool(name="e", bufs=3))
        e2pool = ctx.enter_context(tc.tile_pool(name="e2", bufs=3))
        pspool = ctx.enter_context(tc.tile_pool(name="ps", bufs=1, space="PSUM"))

        rt_sb = singles.tile([P, NCH, M], bf16)
        nc.sync.dma_start(out=rt_sb, in_=rt_d.ap().rearrange("(c p) i -> p c i", p=P))
        sc_sb = singles.tile([P, NCH, NS], f32)
        nc.sync.dma_start(out=sc_sb, in_=sc_d.ap().rearrange("(c p) s -> p c s", p=P))
        mk_sb = singles.tile([NS, M], f32)
        nc.sync.dma_start(out=mk_sb, in_=mk_d.ap())

        # fp8 one-hot lhsT stack (host-built, aligned): oh[:, m, :, :] is a
        # [128, 2, 64] dual-row lhsT whose ones-column lands psum row m.
        oh = singles.tile([P, NS, 2, NS], fp8)
        nc.sync.dma_start(out=oh, in_=oh_d.ap())

        ps = pspool.tile([NS, M], f32)
        nc.vector.memset(ps, 0.0)

        for k in range(32):
            L = 16 * (k + 1)
            e_t = epool.tile([P, 2, NCH, M], bf16, tag="e")
            for slot in range(2):
                m = 2 * k + slot
                peng = nc.gpsimd if PROD_ENGINE(k, slot) == "gps" else nc.vector
                for c in range(NCH):
                    peng.tensor_scalar(
                        out=e_t[:, slot, c, 0:L],
                        in0=rt_sb[:, c, 0:L],
                        scalar1=sc_sb[:, c, m : m + 1],
                        scalar2=0.0,
                        op0=AOT.subtract,
                        op1=AOT.max,
                    )
            e2_t = e2pool.tile([P, 2, NCH, M], fp8, tag="e2")
            nc.scalar.activation(
                out=e2_t[:, :, :, 0:L],
                in_=e_t[:, :, :, 0:L],
                func=AFT.Square,
            )
            for slot in range(2):
                m = 2 * k + slot
                for c2 in range(NCH // 2):
                    nc.tensor.matmul(
                        ps[:, 0:L],
                        oh[:, m, :, :],
                        e2_t[:, slot, 2 * c2 : 2 * c2 + 2, 0:L],
                        start=False,
                        stop=False,
                        skip_group_check=True,
                        perf_mode=mybir.MatmulPerfMode.DoubleRow,
                    )

        masked = singles.tile([NS, M], f32)
        nc.vector.tensor_mul(masked, ps[:, :], mk_sb)
        sqrt_t = singles.tile([NS, M], bf16)
        res = singles.tile([NS, 1], f32)
        nc.scalar.activation(out=sqrt_t, in_=masked, func=AFT.Sqrt, accum_out=res)
        nc.sync.dma_start(out=out_d.ap(), in_=res)

    nc.compile()
    _PROG["nc"] = nc
    return nc


def _in_maps(repr_np, GT_np):
    import ml_dtypes

    r = np.asarray(repr_np, dtype=np.float32)[np.asarray(GT_np).astype(np.int64)]
    rT = np.ascontiguousarray(r.T)  # [N, M] f32
    rT_bf = rT.astype(ml_dtypes.bfloat16)

    ohs = np.zeros((P, NS, 2, NS), dtype=ml_dtypes.float8_e4m3)
    for m in range(NS):
        ohs[:, m, :, m] = 1.0
    ohs = ohs.reshape(P, NS * 2 * NS)

    maps = []
    t_idx = np.arange(M)[None, :]
    for c in range(NCORES):
        s = np.array(_s_list(c))
        sc = np.ascontiguousarray(rT_bf[:, s].astype(np.float32))  # [N, 64]
        mk = (t_idx < s[:, None]).astype(np.float32)  # [64, M]
        maps.append({"rt": rT_bf, "sc": sc, "mk": mk, "oh": ohs})
    return maps


def run_device(repr_np, GT_np, trace=False, trace_cores=None):
    """Run the bass kernel on 8 cores; returns (total, BassKernelResults)."""
    from concourse.bass_utils import run_bass_kernel_spmd

    nc = _build_program()
    maps = _in_maps(repr_np, GT_np)
    res = run_bass_kernel_spmd(
        nc,
        maps,
        core_ids=list(range(NCORES)),
        trace=trace,
        trace_cores=trace_cores,
    )
    total = 0.0
    for core_out in res.results:
        total += float(core_out["out"].astype(np.float64).sum())
    return np.float32(total), res


def kernel(repr, GT):
    total, _ = run_device(repr, GT, trace=False)
    return total
